# revision 1
# baseline (speedup 1.0000x reference)
"""Causal self-attention (B=4, S=2048, C=1024, H=16) on 8 trn2 NeuronCores.

Sharding: core = (batch b in 0..3) x (head-group hg in 0..1), 8 heads/core.
Megatron-style TP: w_qkv column-sharded, w_proj row-sharded per head-group;
each core computes a partial projection output for its batch, host sums the
two partials per batch (collective-free).

Structure (phase-interleaved so qkv matmuls fill attention's ACT-exp gaps):
  A(0,1): v = x W_v (+leading ones col) and qT,kT for pairs 0,1, streamed
          per s-block from shared x tiles (q/k stored bf16 — the qk matmul
          needs K=64, which the fp32r weight path miscomputes on HW)
  B(0):   per sq-block, per sk-chunk-group(2x128):
            scoresT = kT.T@qT (bf16 K=64, head pair row-tiled via
            base_partition); wT = exp(.125*s) (ACT, psum->sbuf, fp32r);
            diagonal groups: wT *= 0/1 causal mask (DVE, AFTER exp so the
            mask is off ACT's critical path);
            outT[65,sq] += v_ext.T @ wT (fp32r; ROW 0 = softmax denom ->
            lane-aligned reciprocal straight from PSUM + partition_broadcast)
  A(2,3) then B(1..3) round-robin by sq-block (3 independent chains keep
          ACT saturated)
  C: out_part = attn_outT.T @ w_proj_rows + b_proj (fp32r)
"""
import numpy as np

import concourse.bass as bass
import concourse.mybir as mybir
import concourse.tile as tile
from concourse import bacc
from concourse.bass_utils import run_bass_kernel_spmd

P = 128
B, S, C, H, D = 4, 2048, 1024, 16, 64
HG = 8                 # heads per core
HD = HG * D            # 512 head dims per core
KC = C // P            # 8 contraction chunks for qkv
SB = 4                 # s blocks of 512
SQ = S // SB           # 512
NEG = -1.0e30

_RUNNER = None

CFG = {
    "psS_bufs": 1,
    "psS_shared": False,   # share one psS tag across the head pair
    "psO_bufs": 1,
    "wt_bufs": 4,
    "rc_bufs": 2,
    "mask_after": True,    # multiplicative 0/1 mask on wT after exp
    "ob_copy": False,      # copy psum out early to release the bank
}


def _build_program():
    nc = bacc.Bacc("TRN2", target_bir_lowering=False)
    f32 = mybir.dt.float32
    f32r = mybir.dt.float32r
    bf16 = mybir.dt.bfloat16

    xT = nc.dram_tensor("xT", [C, S], f32r, kind="ExternalInput")
    wqkv = nc.dram_tensor("wqkv", [C, 3 * HD], f32r, kind="ExternalInput")
    bqk = nc.dram_tensor("bqk", [2 * HD], f32, kind="ExternalInput")
    bv = nc.dram_tensor("bv", [HD], f32, kind="ExternalInput")
    wproj = nc.dram_tensor("wproj", [HD, C], f32r, kind="ExternalInput")
    bproj = nc.dram_tensor("bproj", [C], f32, kind="ExternalInput")
    maskadd = nc.dram_tensor("maskadd", [P, 4, SQ], f32, kind="ExternalInput")
    vones = nc.dram_tensor("vones", [P, HG], f32r, kind="ExternalInput")
    out = nc.dram_tensor("out_part", [S, C], f32, kind="ExternalOutput")

    xT_r = xT[:].rearrange("(kc p) s -> kc p s", p=P)
    wqk_r = wqkv[:, 0:2 * HD].rearrange("(kc p) n -> kc p n", p=P)
    wv_r = wqkv[:, 2 * HD:3 * HD].rearrange("(kc p) n -> kc p n", p=P)

    with tile.TileContext(nc) as tc:
        with (
            tc.tile_pool(name="persist", bufs=1) as pp,
            tc.tile_pool(name="small", bufs=1) as sp,
        ):
            qkT = [
                pp.tile([P, S], bf16, tag=f"qkT{i}", name=f"qkT{i}")
                for i in range(8)
            ]
            v_sb = pp.tile([P, S // P, HG, D + 1], f32r, tag="v_sb")

            bqk_sb = sp.tile([P, 2 * HD // P], f32, tag="bqk")
            nc.sync.dma_start(bqk_sb[:], bqk[:].rearrange("(blk p) -> p blk", p=P))
            bv_bc = sp.tile([P, HD], f32, tag="bv_bc")
            nc.sync.dma_start(bv_bc[:], bv[:].unsqueeze(0).to_broadcast((P, HD)))
            bp_bc = sp.tile([P, C], f32, tag="bp_bc")
            # ones column FIRST in v_ext: denominator lands on psum partition
            # 0 (lane-aligned for a direct reciprocal from PSUM). One clean
            # DMA + 16 strided DVE copies (memset can't write fp32r, and a
            # direct scatter DMA costs ~2048 single-element descriptors).
            ones_sb = sp.tile([P, HG], f32r, tag="ones_sb")
            nc.sync.dma_start(ones_sb[:], vones[:])
            for st in range(S // P):
                nc.vector.tensor_copy(v_sb[:, st, :, 0], ones_sb[:])

            # ---- interleaved A (qkv, v merged into first x sweep) + B ----
            with tc.tile_pool(name="persistBC", bufs=1) as pbc:
                aT = pbc.tile([P, HD // P, S], f32r, tag="attn_outT")
                masks = pbc.tile([P, 4, SQ], f32, tag="masks")

                from contextlib import ExitStack
                stack = ExitStack()
                with stack:
                    xp = stack.enter_context(tc.tile_pool(name="xpool", bufs=2))
                    wp = stack.enter_context(tc.tile_pool(name="wpool", bufs=1))

                    psA1 = stack.enter_context(
                        tc.tile_pool(name="psA1", bufs=2, space="PSUM")
                    )

                    def open_b_pools():
                        wtp = stack.enter_context(
                            tc.tile_pool(name="wtpool", bufs=CFG["wt_bufs"])
                        )
                        psS = stack.enter_context(
                            tc.tile_pool(name="psS", bufs=CFG["psS_bufs"],
                                         space="PSUM")
                        )
                        psO = stack.enter_context(
                            tc.tile_pool(name="psO", bufs=CFG["psO_bufs"],
                                         space="PSUM")
                        )
                        rcp = stack.enter_context(
                            tc.tile_pool(name="rcpool", bufs=CFG["rc_bufs"])
                        )
                        if CFG["mask_after"]:
                            bp["wt2p"] = stack.enter_context(
                                tc.tile_pool(name="wt2pool", bufs=2)
                            )
                        return wtp, psS, psO, rcp

                    vstack = ExitStack()
                    wvp = vstack.enter_context(
                        tc.tile_pool(name="wvpool", bufs=1)
                    )
                    wv_k = [
                        wvp.tile([P, HD], f32r, tag=f"wv{kc}", name=f"wv{kc}")
                        for kc in range(KC)
                    ]
                    for kc in range(KC):
                        nc.sync.dma_start(wv_k[kc][:], wv_r[kc])

                    def emit_a1(pairs, with_v=False):
                        ocs = [hp for hp in pairs] + [4 + hp for hp in pairs]
                        wqk_t = {}

                        def load_wqk():
                            for i, oc in enumerate(ocs):
                                for kc in range(KC):
                                    wt_ = wp.tile([P, P], f32r,
                                                  tag=f"wqk{i}_{kc}",
                                                  name=f"wqk_{oc}_{kc}")
                                    nc.sync.dma_start(
                                        wt_[:],
                                        wqk_r[kc, :, oc * P:(oc + 1) * P],
                                    )
                                    wqk_t[(oc, kc)] = wt_

                        if not with_v:
                            load_wqk()
                        for sb in range(SB):
                            xk = [
                                xp.tile([P, SQ], f32r, tag=f"x{kc}",
                                        name=f"x_{pairs[0]}_{sb}_{kc}")
                                for kc in range(KC)
                            ]
                            for kc in range(KC):
                                nc.sync.dma_start(
                                    xk[kc][:], xT_r[kc, :, sb * SQ:(sb + 1) * SQ]
                                )
                            if with_v:
                                # v natural [s, vcol] from the same x tiles
                                for stl in range(SQ // P):
                                    st = sb * (SQ // P) + stl
                                    ps = psA1.tile([P, HD], f32, tag="psA1",
                                                   name=f"psV_{sb}_{stl}")
                                    for kc in range(KC):
                                        nc.tensor.matmul(
                                            ps[:],
                                            xk[kc][:, stl * P:(stl + 1) * P],
                                            wv_k[kc][:],
                                            start=(kc == 0),
                                            stop=(kc == KC - 1),
                                        )
                                    nc.vector.tensor_add(
                                        out=v_sb[:, st, :, 1:D + 1],
                                        in0=ps[:].rearrange(
                                            "p (h d) -> p h d", h=HG),
                                        in1=bv_bc[:].rearrange(
                                            "p (h d) -> p h d", h=HG),
                                    )
                                if sb == 0:
                                    # weights DMA after sb0's x tiles so the
                                    # first v matmuls aren't starved
                                    load_wqk()
                            for oc in ocs:
                                ps = psA1.tile([P, SQ], f32, tag="psA1")
                                for kc in range(KC):
                                    nc.tensor.matmul(
                                        ps[:],
                                        wqk_t[(oc, kc)][:],
                                        xk[kc][:],
                                        start=(kc == 0),
                                        stop=(kc == KC - 1),
                                    )
                                nc.vector.tensor_scalar_add(
                                    qkT[oc][:, sb * SQ:(sb + 1) * SQ],
                                    ps[:],
                                    bqk_sb[:, oc:oc + 1],
                                )

                    bp = {}

                    def emit_b_unit(hp, j):
                        psO = bp["psO"]
                        qT_blk = qkT[hp]
                        kT_blk = qkT[4 + hp]
                        if True:
                            ngrp = 2 * (j + 1)
                            sq = slice(j * SQ, (j + 1) * SQ)
                            po = [
                                psO.tile([D + 1, SQ], f32, tag=f"psO{h}",
                                         name=f"psO_{hp}_{j}_{h}")
                                for h in range(2)
                            ]
                            for g in range(ngrp):
                                for h in range(2):
                                    p0 = h * D
                                    pss = psS.tile(
                                        [P, 2, SQ], f32,
                                        tag="psS" if CFG["psS_shared"]
                                        else f"psS{h}",
                                        name=f"psS_{hp}_{j}_{g}_{h}",
                                    )
                                    for u in range(2):
                                        t = 2 * g + u
                                        nc.tensor.matmul(
                                            pss[:, u, :],
                                            kT_blk[p0:p0 + D,
                                                   t * P:(t + 1) * P],
                                            qT_blk[p0:p0 + D, sq],
                                            start=True,
                                            stop=True,
                                        )
                                    diag = g >= ngrp - 2
                                    mg = 2 * (g - (ngrp - 2))
                                    if diag and not CFG["mask_after"]:
                                        nc.vector.tensor_add(
                                            out=pss[:],
                                            in0=pss[:],
                                            in1=masks[:, mg:mg + 2, :],
                                        )
                                    wT = wtp.tile([P, 2, SQ], f32r, tag="wT")
                                    nc.scalar.activation(
                                        wT[:], pss[:],
                                        mybir.ActivationFunctionType.Exp,
                                        scale=0.125,
                                    )
                                    if diag and CFG["mask_after"]:
                                        # 0/1 mask on the exp'd weights keeps
                                        # the DVE op off ACT's critical path
                                        wT2 = bp["wt2p"].tile(
                                            [P, 2, SQ], f32r, tag="wT2")
                                        nc.vector.tensor_mul(
                                            out=wT2[:],
                                            in0=wT[:],
                                            in1=masks[:, mg:mg + 2, :]
                                            .bitcast(f32r),
                                        )
                                        wT = wT2
                                    for u in range(2):
                                        t = 2 * g + u
                                        nc.tensor.matmul(
                                            po[h][:],
                                            v_sb[:, t, hp * 2 + h, :],
                                            wT[:, u, :],
                                            start=(t == 0),
                                            stop=(t == 4 * (j + 1) - 1),
                                        )
                            for h in range(2):
                                # denom on psum partition 0 (ones col first):
                                # direct lane-aligned reciprocal from PSUM
                                if CFG["ob_copy"]:
                                    src = rcp.tile([D + 1, SQ], f32, tag="ob")
                                    nc.vector.tensor_copy(src[:], po[h][:])
                                else:
                                    src = po[h]
                                rc = rcp.tile([1, SQ], f32, tag="rc")
                                nc.vector.reciprocal(rc[:], src[0:1, :])
                                rcb = rcp.tile([D + 1, SQ], f32, tag="rcb")
                                nc.gpsimd.partition_broadcast(rcb[:], rc[:])
                                # engines need 32-aligned partition bases:
                                # multiply all 65 rows (row 0 harmless), DMA
                                # extracts rows 1..64
                                nt = rcp.tile([D + 1, SQ], f32r, tag="nt")
                                nc.vector.tensor_mul(
                                    out=nt[:], in0=src[:], in1=rcb[:],
                                )
                                nc.sync.dma_start(
                                    aT[h * D:(h + 1) * D, hp, sq],
                                    nt[1:D + 1, :],
                                )

                    emit_a1((0, 1), with_v=True)
                    # non-critical loads emitted after the startup-critical
                    # x/wv/wqk stream: masks gate only B's diagonal groups,
                    # bp_bc only phase C
                    nc.sync.dma_start(masks[:], maskadd[:])
                    nc.sync.dma_start(
                        bp_bc[:], bproj[:].unsqueeze(0).to_broadcast((P, C))
                    )
                    vstack.close()  # free wv weights before B pools open
                    bp["psO"] = None
                    wtp, psS, psO, rcp = open_b_pools()
                    bp["psO"] = psO
                    for j in range(SB):
                        for pair in (0, 1):
                            emit_b_unit(pair, j)
                    emit_a1((2, 3))
                    for j in range(SB):
                        for pair in (2, 3):
                            emit_b_unit(pair, j)

                # ---- Phase C: projection ----
                with (
                    tc.tile_pool(name="wppool", bufs=1) as wpp,
                    tc.tile_pool(name="opool", bufs=4) as op,
                    tc.tile_pool(name="psC", bufs=2, space="PSUM") as psC,
                ):
                    wp_sb = wpp.tile([P, HD // P, C], f32r, tag="wp_sb")
                    nc.sync.dma_start(
                        wp_sb[:], wproj[:].rearrange("(hp p) n -> p hp n", p=P)
                    )
                    for st in range(S // P):
                        for ocb in range(2):
                            nsl = slice(ocb * SQ, (ocb + 1) * SQ)
                            ps = psC.tile([P, SQ], f32, tag="psC")
                            for hp in range(HD // P):
                                nc.tensor.matmul(
                                    ps[:],
                                    aT[:, hp, st * P:(st + 1) * P],
                                    wp_sb[:, hp, nsl],
                                    start=(hp == 0),
                                    stop=(hp == HD // P - 1),
                                )
                            ot = op.tile([P, SQ], f32, tag="ot")
                            nc.vector.tensor_add(
                                out=ot[:], in0=ps[:], in1=bp_bc[:, nsl]
                            )
                            nc.sync.dma_start(out[st * P:(st + 1) * P, nsl], ot[:])

    nc.compile()
    return nc


def _make_masks():
    # scoresT diagonal chunk t (of 4): [sk p, sq f], allowed iff f >= t*128+p
    # mask_after: multiplicative 0/1 applied to exp(s); else additive -1e30
    m = np.zeros((4, P, SQ), np.float32)
    f = np.arange(SQ)[None, :]
    p = np.arange(P)[:, None]
    on, off = (1.0, 0.0) if CFG["mask_after"] else (0.0, NEG)
    for t in range(4):
        m[t] = np.where(f >= t * P + p, on, off)
    return np.ascontiguousarray(m.transpose(1, 0, 2))  # [128, 4, 512]


def _shard_inputs(x, w_qkv, b_qkv, w_proj, b_proj):
    masks = _make_masks()
    x = np.asarray(x, np.float32)
    w_qkv = np.asarray(w_qkv, np.float32)
    b_qkv = np.asarray(b_qkv, np.float32)
    w_proj = np.asarray(w_proj, np.float32)
    b_proj = np.asarray(b_proj, np.float32)
    zeros_c = np.zeros((C,), np.float32)
    in_maps = []
    for core in range(8):
        b, hg = core // 2, core % 2
        cs = slice(hg * HD, (hg + 1) * HD)
        wq = w_qkv[:, 0:C][:, cs]
        wk = w_qkv[:, C:2 * C][:, cs]
        wv = w_qkv[:, 2 * C:3 * C][:, cs]
        bq = b_qkv[0:C][cs]
        bk = b_qkv[C:2 * C][cs]
        bvv = b_qkv[2 * C:3 * C][cs]
        in_maps.append({
            "xT": np.ascontiguousarray(x[b].T),
            "wqkv": np.ascontiguousarray(np.concatenate([wq, wk, wv], axis=1)),
            "bqk": np.ascontiguousarray(np.concatenate([bq, bk])),
            "bv": np.ascontiguousarray(bvv),
            "wproj": np.ascontiguousarray(w_proj[cs, :]),
            "bproj": b_proj if hg == 0 else zeros_c,
            "maskadd": masks,
            "vones": np.ones((P, HG), np.float32),
        })
    return in_maps


def get_program():
    global _RUNNER
    if _RUNNER is None:
        _RUNNER = _build_program()
    return _RUNNER


def kernel(x, w_qkv, b_qkv, w_proj, b_proj):
    nc = get_program()
    in_maps = _shard_inputs(x, w_qkv, b_qkv, w_proj, b_proj)
    res = run_bass_kernel_spmd(nc, in_maps, list(range(8)))
    out = np.empty((B, S, C), np.float32)
    for b in range(B):
        out[b] = res.results[2 * b]["out_part"] + res.results[2 * b + 1]["out_part"]
    return out

